# revision 1
# baseline (speedup 1.0000x reference)
"""GAT (2-layer) Trainium2 Bass kernel — 8-core SPMD.

Sharding: dst nodes across 8 cores (12500 each). Per core, dsts are packed
into 98 windows of 128 (one SBUF partition per dst), profile-sorted so slot
padding is small. Edge rows [a_s f32 | h bf16] are fetched by
gpsimd.dma_gather from per-layer node tables (4 src-groups of 25088 rows to
fit int16 indices); attention + weighted segment-sum run as per-partition
DVE ops. Pad slots point at a row with a_s=-300 => weights ~e^-56, no
masking needed. Layer-2 node table is built per-shard and AllGathered.
"""

import numpy as np
import ml_dtypes

import concourse.bacc as bacc
import concourse.bass as bass
import concourse.mybir as mybir
import concourse.tile as tile
from concourse.bass_utils import run_bass_kernel_spmd
from concourse.masks import make_identity

F32 = mybir.dt.float32
BF16 = mybir.dt.bfloat16
I16 = mybir.dt.int16
AX = mybir.AxisListType
OP = mybir.AluOpType
ACT = mybir.ActivationFunctionType

N, E = 100000, 1600000
IN, HID, OUT, HEADS = 256, 16, 64, 8
NEG = 0.2
NCORES = 8
NSH = N // NCORES        # 12500
NGRP = 4
GSZ = N // NGRP          # 25000
NP = 25088               # padded rows per group (196*128)
NW = 98                  # windows per core
SH_ROWS = NW * 128       # 12544
PAD1 = GSZ               # group-local pad row, table1 (25000; rows 25000..25087 zero-x)
PAD2 = NSH               # group-local pad row, table2 (12500 in core 2g's shard)
COLS_BUDGET = 80         # slot columns per gather batch
ROW1 = 256               # bf16 elems per table1 row (512B)
ROW2 = 128               # bf16 elems per table2 row (256B)
A_S_NEG = -300.0


# ---------------------------------------------------------------- host side
def _layout(src, dst):
    core = dst // NSH
    grp = src // GSZ
    cg_all = np.zeros((NCORES, NSH, NGRP), np.int32)
    np.add.at(cg_all, (core, dst % NSH, grp), 1)
    perms = []
    for k in range(NCORES):
        cg = cg_all[k]
        perms.append(np.lexsort((cg[:, 3], cg[:, 2], cg[:, 1], cg[:, 0]))[::-1])
    Lg = np.zeros((NW, NGRP), np.int64)
    for k in range(NCORES):
        cgp = cg_all[k][perms[k]]
        cgp = np.concatenate([cgp, np.zeros((SH_ROWS - NSH, NGRP), np.int32)])
        Lg = np.maximum(Lg, cgp.reshape(NW, 128, NGRP).max(axis=1))
    Lw = Lg.sum(axis=1)
    sig = np.empty(N, np.int64)
    for k in range(NCORES):
        pos = np.empty(NSH, np.int64)
        pos[perms[k]] = np.arange(NSH)
        sig[k * NSH:(k + 1) * NSH] = k * SH_ROWS + pos
    eorder = np.lexsort((grp, dst))
    es, ed, eg, ec = src[eorder], dst[eorder], grp[eorder], core[eorder]
    core_starts = np.searchsorted(ec, np.arange(NCORES + 1))
    cores = [(es[a:b], (ed[a:b] - k * NSH), eg[a:b])
             for k, (a, b) in enumerate(zip(core_starts[:-1], core_starts[1:]))]
    return dict(Lg=Lg, Lw=Lw, perms=perms, sig=sig, cores=cores)


def _pack_idx(arr_pj):
    """[128, cols] slot-array of indices -> wrapped idx tile [128, cols*8]."""
    I = arr_pj.T.ravel()                      # I[j*128+p]
    W = I.reshape(-1, 16).T.astype(np.int16)  # [16, len/16]
    return np.tile(W, (8, 1))


def _host_inputs(inputs, lay, batches):
    x = np.asarray(inputs["x"], np.float32)
    W1 = np.asarray(inputs["W1"], np.float64)
    att1_s = np.asarray(inputs["att1_s"], np.float64)
    att1_d = np.asarray(inputs["att1_d"], np.float64)
    W2 = np.asarray(inputs["W2"], np.float64)
    att2_s = np.asarray(inputs["att2_s"], np.float64)
    att2_d = np.asarray(inputs["att2_d"], np.float64)
    b1 = np.asarray(inputs["b1"], np.float32)
    b2 = np.asarray(inputs["b2"], np.float32)
    Lg, Lw, perms, sig = lay["Lg"], lay["Lw"], lay["perms"], lay["sig"]

    A_s = np.zeros((HEADS * HID, HEADS))
    A_d = np.zeros((HEADS * HID, HEADS))
    for h in range(HEADS):
        A_s[h * HID:(h + 1) * HID, h] = att1_s[h]
        A_d[h * HID:(h + 1) * HID, h] = att1_d[h]
    w1r = np.concatenate([W1, W1 @ A_s, W1 @ A_d], axis=1)          # [256,144]
    w2r = np.concatenate([W2, W2 @ att2_s.T, W2 @ att2_d.T], axis=1)  # [128,66]
    w1r_bf = w1r.astype(ml_dtypes.bfloat16)
    w2r_bf = w2r.astype(ml_dtypes.bfloat16)

    xT = np.zeros((IN, NGRP * NP), np.float32)
    for g in range(NGRP):
        xT[:, g * NP:g * NP + GSZ] = x[g * GSZ:(g + 1) * GSZ].T
    xT_bf = xT.astype(ml_dtypes.bfloat16)

    common = {
        "xt0": np.ascontiguousarray(xT_bf[:128]),
        "xt1": np.ascontiguousarray(xT_bf[128:]),
        "w1r0": np.ascontiguousarray(w1r_bf[:128]),
        "w1r1": np.ascontiguousarray(w1r_bf[128:]),
        "w2r": np.ascontiguousarray(w2r_bf),
        "b1rep": np.ascontiguousarray(np.tile(b1[None, :], (128, 1)).astype(np.float32)),
        "b2rep": np.ascontiguousarray(np.tile(b2[None, :], (128, 1)).astype(np.float32)),
    }

    per_core = []
    for k in range(NCORES):
        es, edl, eg = lay["cores"][k]
        pos = np.empty(NSH, np.int64)
        pos[perms[k]] = np.arange(NSH)
        o = np.lexsort((eg, pos[edl]))
        es_o, eg_o, pos_o = es[o], eg[o], pos[edl][o]
        w_o, p_o = pos_o // 128, pos_o % 128
        key = pos_o * NGRP + eg_o
        slot = np.arange(len(o)) - np.searchsorted(key, key)
        idx1_secs, idx2_secs = [], []
        for ws in batches:
            for g in range(NGRP):
                cols = int(Lg[ws, g].sum())
                if cols == 0:
                    continue
                a1 = np.full((128, cols), PAD1, np.int64)
                a2 = np.full((128, cols), PAD2, np.int64)
                coff = 0
                for w in ws:
                    m = (w_o == w) & (eg_o == g)
                    pp, jj, ss = p_o[m], slot[m], es_o[m]
                    a1[pp, coff + jj] = ss % GSZ
                    a2[pp, coff + jj] = sig[ss] % NP
                    coff += int(Lg[w, g])
                idx1_secs.append(a1)
                idx2_secs.append(a2)
        idx1 = np.concatenate([_pack_idx(a) for a in idx1_secs], axis=1)
        idx2 = np.concatenate([_pack_idx(a) for a in idx2_secs], axis=1)
        xtp = np.zeros((IN, SH_ROWS), np.float32)
        xtp[:, :NSH] = x[k * NSH:(k + 1) * NSH].T[:, perms[k]]
        xtp_bf = xtp.astype(ml_dtypes.bfloat16)
        d = dict(common)
        d["idx1"] = np.ascontiguousarray(idx1)
        d["idx2"] = np.ascontiguousarray(idx2)
        d["xtp0"] = np.ascontiguousarray(xtp_bf[:128])
        d["xtp1"] = np.ascontiguousarray(xtp_bf[128:])
        per_core.append(d)
    return per_core


# ------------------------------------------------------------- device side
def _build_program(Lg, Lw, batches):
    nc = bacc.Bacc("TRN2", target_bir_lowering=False, debug=False,
                   num_devices=NCORES)
    IDXF = int(Lg.sum()) * 8
    LWMAX = int(Lw.max())
    MAXC = max(COLS_BUDGET, LWMAX)
    xt0 = nc.declare_dram_parameter("xt0", [128, NGRP * NP], BF16, isOutput=False)
    xt1 = nc.declare_dram_parameter("xt1", [128, NGRP * NP], BF16, isOutput=False)
    w1r0 = nc.declare_dram_parameter("w1r0", [128, 144], BF16, isOutput=False)
    w1r1 = nc.declare_dram_parameter("w1r1", [128, 144], BF16, isOutput=False)
    w2r = nc.declare_dram_parameter("w2r", [128, 66], BF16, isOutput=False)
    b1rep = nc.declare_dram_parameter("b1rep", [128, 128], F32, isOutput=False)
    b2rep = nc.declare_dram_parameter("b2rep", [128, 64], F32, isOutput=False)
    idx1 = nc.declare_dram_parameter("idx1", [128, IDXF], I16, isOutput=False)
    idx2 = nc.declare_dram_parameter("idx2", [128, IDXF], I16, isOutput=False)
    xtp0 = nc.declare_dram_parameter("xtp0", [128, SH_ROWS], BF16, isOutput=False)
    xtp1 = nc.declare_dram_parameter("xtp1", [128, SH_ROWS], BF16, isOutput=False)
    outp = nc.declare_dram_parameter("out", [SH_ROWS, OUT], F32, isOutput=True)

    table1 = nc.dram_tensor("table1", [NGRP * NP, ROW1], BF16)
    shard2 = nc.dram_tensor("shard2", [SH_ROWS, ROW2], BF16)
    table2 = nc.dram_tensor("table2", [NCORES * SH_ROWS, ROW2], BF16)

    dma_sem = nc.alloc_semaphore("g_dma")
    prep_sem = nc.alloc_semaphore("g_prep")
    cc_sem = nc.alloc_semaphore("cc")
    gn = [0]

    TPB, BLK, WRB = 196, 14, 7

    with tile.TileContext(nc) as tc:
        with (
            tc.tile_pool(name="const", bufs=1) as constp,
            tc.tile_pool(name="xt", bufs=2) as xtpool,
            tc.tile_pool(name="dense", bufs=2) as densep,
            tc.tile_pool(name="psum", bufs=2, space="PSUM") as psump,
            tc.tile_pool(name="stag", bufs=2) as stagp,
            tc.tile_pool(name="idx", bufs=2) as idxp,
            tc.tile_pool(name="work", bufs=2) as workp,
            tc.tile_pool(name="small", bufs=3) as smallp,
        ):
            w1r0_t = constp.tile([128, 144], BF16, tag="w1r0")
            w1r1_t = constp.tile([128, 144], BF16, tag="w1r1")
            w2r_t = constp.tile([128, 66], BF16, tag="w2r")
            b1_t = constp.tile([128, 128], F32, tag="b1")
            b2_t = constp.tile([128, 64], F32, tag="b2")
            ident = constp.tile([128, 128], BF16, tag="ident")
            adwin = constp.tile([128, NW * HEADS], F32, tag="adwin")
            ad2win = constp.tile([128, NW], F32, tag="ad2win")
            nc.sync.dma_start(out=w1r0_t[:], in_=w1r0[:])
            nc.sync.dma_start(out=w1r1_t[:], in_=w1r1[:])
            nc.sync.dma_start(out=w2r_t[:], in_=w2r[:])
            nc.sync.dma_start(out=b1_t[:], in_=b1rep[:])
            nc.sync.dma_start(out=b2_t[:], in_=b2rep[:])
            make_identity(nc, ident[:])

            # ---------------- phase 0: dense h1 table (all nodes) ----------
            for g in range(NGRP):
                for blk in range(TPB // BLK):
                    base = g * NP + blk * BLK * 128
                    xs0 = xtpool.tile([128, BLK * 128], BF16, tag="xs0")
                    xs1 = xtpool.tile([128, BLK * 128], BF16, tag="xs1")
                    nc.sync.dma_start(out=xs0[:], in_=xt0[:, base:base + BLK * 128])
                    nc.sync.dma_start(out=xs1[:], in_=xt1[:, base:base + BLK * 128])
                    for wb in range(BLK // WRB):
                        rows = densep.tile([128, WRB * ROW1], BF16, tag="rows")
                        for t in range(WRB):
                            tt = wb * WRB + t
                            ps = psump.tile([128, 144], F32, tag="ps0")
                            nc.tensor.matmul(
                                out=ps[:], lhsT=xs0[:, tt * 128:(tt + 1) * 128],
                                rhs=w1r0_t[:], start=True, stop=False)
                            nc.tensor.matmul(
                                out=ps[:], lhsT=xs1[:, tt * 128:(tt + 1) * 128],
                                rhs=w1r1_t[:], start=False, stop=True)
                            rv = rows[:, t * ROW1:(t + 1) * ROW1]
                            nc.vector.tensor_copy(out=rv[0:128, 0:16].bitcast(F32),
                                                  in_=ps[:, 128:136])
                            nc.vector.tensor_copy(out=rv[0:128, 16:144],
                                                  in_=ps[:, 0:128])
                        wbase = g * NP + (blk * BLK + wb * WRB) * 128
                        nc.sync.dma_start(
                            out=table1[wbase:wbase + WRB * 128, :]
                                .rearrange("(a p) r -> p a r", p=128),
                            in_=rows[:].rearrange("p (a r) -> p a r", a=WRB))
            # pad row: a_s := -300 (h stays 0) on group-local row PAD1
            padrow = constp.tile([128, ROW1], BF16, tag="padrow")
            nc.vector.memset(padrow[:], 0.0)
            nc.vector.memset(padrow[0:1, 0:16].bitcast(F32), A_S_NEG)
            for g in range(NGRP):
                nc.sync.dma_start(out=table1[g * NP + PAD1:g * NP + PAD1 + 1, :],
                                  in_=padrow[0:1, :])

            # a_d per window (window-ordered x.T)
            for w in range(NW):
                xp0 = xtpool.tile([128, 128], BF16, tag="xp0")
                xp1 = xtpool.tile([128, 128], BF16, tag="xp1")
                nc.sync.dma_start(out=xp0[:], in_=xtp0[:, w * 128:(w + 1) * 128])
                nc.sync.dma_start(out=xp1[:], in_=xtp1[:, w * 128:(w + 1) * 128])
                psa = psump.tile([128, 16], F32, tag="psa")
                nc.tensor.matmul(out=psa[:], lhsT=xp0[:], rhs=w1r0_t[:, 128:144],
                                 start=True, stop=False)
                nc.tensor.matmul(out=psa[:], lhsT=xp1[:], rhs=w1r1_t[:, 128:144],
                                 start=False, stop=True)
                nc.vector.tensor_copy(out=adwin[:, w * 8:(w + 1) * 8],
                                      in_=psa[:, 8:16])

            # ---------------- edge layers ----------------------------------
            import os as _os

            def edge_layer(layer):
                import os
                NBATCH = int(os.environ.get("GAT_NBATCH", "999"))
                tabl, row_e = (table1, ROW1) if layer == 1 else (table2, ROW2)
                idxin = idx1 if layer == 1 else idx2
                nh = HEADS if layer == 1 else 1
                nch = HID if layer == 1 else OUT
                hoff = 16 if layer == 1 else 2
                idx_off = 0

                for ws in batches[:NBATCH]:
                    cols_b = int(Lw[ws].sum())
                    stag = stagp.tile([128, MAXC * ROW1], BF16, tag="st")
                    gbase = np.concatenate([[0], np.cumsum(
                        [int(Lg[ws, g].sum()) for g in range(NGRP)])])
                    for g in range(NGRP):
                        cols = int(Lg[ws, g].sum())
                        if cols == 0:
                            continue
                        nidx = 128 * cols
                        ixt = idxp.tile([128, MAXC * 8], I16, tag="ix")
                        nc.sync.dma_start(
                            out=ixt[:, 0:nidx // 16],
                            in_=idxin[:, idx_off:idx_off + nidx // 16])
                        idx_off += nidx // 16
                        sl = stag[:, int(gbase[g]) * row_e:
                                  (int(gbase[g]) + cols) * row_e]
                        sl3 = sl.rearrange("p (k d) -> p k d", d=row_e)
                        gn[0] += 1
                        n_g = gn[0]
                        if not int(os.environ.get("GAT_GATHER", "1")):
                            gn[0] -= 1
                            continue
                        with tc.tile_critical():
                            nc.gpsimd.dma_gather(
                                out_ap=sl3, in_ap=tabl[g * NP:(g + 1) * NP, :],
                                idxs_ap=ixt[:, 0:nidx // 16],
                                num_idxs=nidx, num_idxs_reg=nidx,
                                elem_size=row_e, single_packet=False,
                                prepare_only=True, sem=dma_sem,
                            ).then_inc(prep_sem, 1)
                            nc.gpsimd.wait_ge(prep_sem, n_g)
                            nc.gpsimd.trigger_dma(count=1)
                            nc.gpsimd.wait_ge(dma_sem, 16 * n_g)
                            if int(os.environ.get("GAT_ANCHOR", "1")):
                                nc.gpsimd.tensor_copy(out=sl3[:, :, 0:1],
                                                      in_=sl3[:, :, 0:1])
                    woff = np.zeros(NGRP, np.int64)
                    if not int(os.environ.get("GAT_COMPUTE", "1")):
                        continue
                    for w in ws:
                        Lwv = int(Lw[w])
                        wall_t = workp.tile([128, LWMAX * HEADS], F32, tag="wa")
                        wall = wall_t[:, 0:Lwv * nh]
                        msg_t = workp.tile([128, LWMAX * HEADS * HID], BF16, tag="mg")
                        msg = msg_t[:, 0:Lwv * nh * nch]
                        wsec = 0
                        for g in range(NGRP):
                            Lgv = int(Lg[w, g])
                            if Lgv == 0:
                                continue
                            c0 = int(gbase[g] + woff[g])
                            sl3 = stag[:, c0 * row_e:(c0 + Lgv) * row_e] \
                                .rearrange("p (l r) -> p l r", l=Lgv)
                            a_s = sl3[:, :, 0:2 * nh].bitcast(F32)
                            if layer == 1:
                                adv = adwin[:, w * 8:(w + 1) * 8]
                            else:
                                adv = ad2win[:, w:w + 1]
                            adv = adv.rearrange("p (l h) -> p l h", l=1) \
                                .to_broadcast([128, Lgv, nh])
                            uv = wall_t[:, wsec * nh:(wsec + Lgv) * nh] \
                                .rearrange("p (l h) -> p l h", l=Lgv)
                            nc.vector.tensor_tensor(out=uv, in0=a_s, in1=adv,
                                                    op=OP.add)
                            wsec += Lgv
                        # lrelu + exp
                        lr = workp.tile([128, LWMAX * HEADS], F32, tag="lr")
                        nc.vector.tensor_scalar_mul(lr[:, 0:Lwv * nh], wall, NEG)
                        nc.vector.tensor_tensor(out=wall, in0=wall,
                                                in1=lr[:, 0:Lwv * nh], op=OP.max)
                        nc.scalar.activation(wall, wall, ACT.Exp, 0.0, 1.0)
                        # weighted messages
                        wsec = 0
                        for g in range(NGRP):
                            Lgv = int(Lg[w, g])
                            if Lgv == 0:
                                continue
                            c0 = int(gbase[g] + woff[g])
                            sl3 = stag[:, c0 * row_e:(c0 + Lgv) * row_e] \
                                .rearrange("p (l r) -> p l r", l=Lgv)
                            hv = sl3[:, :, hoff:hoff + nh * nch] \
                                .rearrange("p l (h c) -> p l h c", h=nh)
                            wv = wall_t[:, wsec * nh:(wsec + Lgv) * nh] \
                                .rearrange("p (l h c) -> p l h c", l=Lgv, h=nh, c=1) \
                                .to_broadcast([128, Lgv, nh, nch])
                            mv = msg_t[:, wsec * nh * nch:(wsec + Lgv) * nh * nch] \
                                .rearrange("p (l h c) -> p l h c", l=Lgv, h=nh)
                            nc.vector.tensor_tensor(out=mv, in0=hv, in1=wv,
                                                    op=OP.mult)
                            wsec += Lgv
                            woff[g] += Lgv
                        den = smallp.tile([128, HEADS], F32, tag="den")
                        nc.vector.tensor_reduce(
                            out=den[:, 0:nh],
                            in_=wall.rearrange("p (l h) -> p h l", l=Lwv),
                            axis=AX.X, op=OP.add)
                        opre = smallp.tile([128, HEADS * HID], F32, tag="opre")
                        nc.vector.tensor_reduce(
                            out=opre[:, 0:nh * nch],
                            in_=msg.rearrange("p (l h c) -> p h c l", l=Lwv, h=nh),
                            axis=AX.X, op=OP.add)
                        nc.vector.tensor_scalar_max(den[:, 0:nh], den[:, 0:nh], 1e-30)
                        rec = smallp.tile([128, HEADS], F32, tag="rec")
                        nc.vector.reciprocal(rec[:, 0:nh], den[:, 0:nh])
                        o1 = smallp.tile([128, HEADS * HID], F32, tag="o1")
                        nc.vector.tensor_tensor(
                            out=o1[:, 0:nh * nch].rearrange("p (h c) -> p h c", h=nh),
                            in0=opre[:, 0:nh * nch].rearrange("p (h c) -> p h c", h=nh),
                            in1=rec[:, 0:nh].rearrange("p (h c) -> p h c", c=1)
                                .to_broadcast([128, nh, nch]),
                            op=OP.mult)
                        if layer == 1:
                            nc.vector.tensor_tensor(out=o1[:], in0=o1[:],
                                                    in1=b1_t[:], op=OP.add)
                            tneg = smallp.tile([128, 128], F32, tag="tneg")
                            nc.vector.tensor_scalar_min(tneg[:], o1[:], 0.0)
                            nc.scalar.activation(tneg[:], tneg[:], ACT.Exp, 0.0, 1.0)
                            nc.vector.tensor_relu(o1[:], o1[:])
                            nc.vector.tensor_tensor(out=o1[:], in0=o1[:],
                                                    in1=tneg[:], op=OP.add)
                            nc.vector.tensor_scalar_add(o1[:], o1[:], -1.0)
                            o1bf = smallp.tile([128, 128], BF16, tag="o1bf")
                            nc.vector.tensor_copy(out=o1bf[:], in_=o1[:])
                            pst = psump.tile([128, 128], BF16, tag="pst")
                            nc.tensor.transpose(out=pst[:], in_=o1bf[:],
                                                identity=ident[:])
                            o1T = smallp.tile([128, 128], BF16, tag="o1T")
                            nc.vector.tensor_copy(out=o1T[:], in_=pst[:])
                            ps2 = psump.tile([128, 66], F32, tag="ps2")
                            nc.tensor.matmul(out=ps2[:], lhsT=o1T[:], rhs=w2r_t[:],
                                             start=True, stop=True)
                            row2 = smallp.tile([128, ROW2], BF16, tag="row2")
                            nc.vector.tensor_copy(out=row2[0:128, 0:2].bitcast(F32),
                                                  in_=ps2[:, 64:65])
                            nc.vector.tensor_copy(out=row2[0:128, 2:66],
                                                  in_=ps2[:, 0:64])
                            nc.vector.tensor_copy(out=ad2win[:, w:w + 1],
                                                  in_=ps2[:, 65:66])
                            nc.sync.dma_start(out=shard2[w * 128:(w + 1) * 128, :],
                                              in_=row2[:])
                        else:
                            nc.vector.tensor_tensor(out=o1[:, 0:64], in0=o1[:, 0:64],
                                                    in1=b2_t[:], op=OP.add)
                            mx = smallp.tile([128, 1], F32, tag="mx")
                            nc.vector.tensor_reduce(out=mx[:], in_=o1[:, 0:64],
                                                    axis=AX.X, op=OP.max)
                            sh = smallp.tile([128, 64], F32, tag="sh")
                            nc.vector.tensor_scalar(
                                out=sh[:], in0=o1[:, 0:64], scalar1=mx[:, 0:1],
                                scalar2=None, op0=OP.subtract)
                            ex = smallp.tile([128, 64], F32, tag="ex")
                            nc.scalar.activation(ex[:], sh[:], ACT.Exp, 0.0, 1.0)
                            se = smallp.tile([128, 1], F32, tag="se")
                            nc.vector.tensor_reduce(out=se[:], in_=ex[:],
                                                    axis=AX.X, op=OP.add)
                            ln = smallp.tile([128, 1], F32, tag="ln")
                            nc.scalar.activation(ln[:], se[:], ACT.Ln, 0.0, 1.0)
                            fo = smallp.tile([128, 64], F32, tag="fo")
                            nc.vector.tensor_scalar(
                                out=fo[:], in0=sh[:], scalar1=ln[:, 0:1],
                                scalar2=None, op0=OP.subtract)
                            nc.sync.dma_start(out=outp[w * 128:(w + 1) * 128, :],
                                              in_=fo[:])

            STAGE = int(_os.environ.get("GAT_STAGE", "3"))
            if STAGE == 0:
                # debug: dump a table1 slice
                dbg = smallp.tile([128, 64], F32, tag="dbg")
                t1v = stagp.tile([128, ROW1], BF16, tag="st")
                nc.sync.dma_start(out=t1v[:], in_=table1[0:128, :])
                nc.vector.tensor_copy(out=dbg[:], in_=t1v[:, 16:80])
                for w in range(NW):
                    nc.sync.dma_start(out=outp[w * 128:(w + 1) * 128, :], in_=dbg[:])
            if STAGE >= 1:
                edge_layer(1)
            if STAGE == 1:
                if int(_os.environ.get("GAT_DUMP", "1")):
                    dbg = smallp.tile([128, 64], F32, tag="dbg")
                    s2v = stagp.tile([128, ROW2], BF16, tag="st")
                    for w in range(NW):
                        nc.sync.dma_start(out=s2v[:], in_=shard2[w * 128:(w + 1) * 128, :])
                        nc.vector.tensor_copy(out=dbg[:], in_=s2v[:, 2:66])
                        nc.sync.dma_start(out=outp[w * 128:(w + 1) * 128, :], in_=dbg[:])
            pr2 = constp.tile([1, 2], BF16, tag="pr2")
            nc.vector.memset(pr2[0:1, 0:2].bitcast(F32), A_S_NEG)
            nc.sync.dma_start(out=shard2[PAD2:PAD2 + 1, 0:2], in_=pr2[0:1, :])
            if STAGE >= 2:
                with tc.tile_critical():
                    nc.gpsimd.collective_compute(
                        "AllGather", OP.bypass,
                        replica_groups=[list(range(NCORES))],
                        ins=[shard2[:]], outs=[table2[:]],
                    ).then_inc(cc_sem, 1)
                    nc.gpsimd.wait_ge(cc_sem, 1)
            if STAGE >= 3:
                edge_layer(2)
    nc.compile()
    return nc


_CACHE = {}


def kernel(**inputs):
    ei = np.asarray(inputs["edge_index"])
    src, dst = ei[0].astype(np.int64), ei[1].astype(np.int64)
    lay = _layout(src, dst)
    batches = []
    cur, acc = [], 0
    for w in range(NW):
        lw = int(lay["Lw"][w])
        if cur and (acc + lw > COLS_BUDGET or len(cur) >= 8):
            batches.append(cur); cur, acc = [], 0
        cur.append(w); acc += lw
    if cur:
        batches.append(cur)
    per_core = _host_inputs(inputs, lay, batches)
    key = (ei.tobytes()[:64], int(lay["Lg"].sum()))
    if key not in _CACHE:
        _CACHE[key] = _build_program(lay["Lg"], lay["Lw"], batches)
    nc = _CACHE[key]
    res = run_bass_kernel_spmd(nc, per_core, core_ids=list(range(NCORES)))
    out = np.empty((N, OUT), np.float32)
    for k in range(NCORES):
        out[k * NSH + lay["perms"][k]] = res.results[k]["out"][:NSH]
    return out


if __name__ == "__main__":
    d = np.load("/root/problem/inputs.npz")
    o = kernel(**{k: d[k] for k in d.files})
    ref = np.load("/root/problem/ref_out.npy")
    rel = np.linalg.norm(o - ref) / np.linalg.norm(ref)
    err = np.abs(o - ref) / (np.abs(ref) + 1e-5)
    print("fro rel err:", rel, "max elem rel err:", err.max())



# revision 2
# speedup vs baseline: 1.0816x; 1.0816x over previous
"""GAT (2-layer) Trainium2 Bass kernel — 8-core SPMD, v2.

Sharding: dst nodes across 8 cores (12500 each), packed into 98 windows of
128 (one SBUF partition per dst), profile-sorted. Edge rows (256B: h only)
are fetched by gpsimd.dma_gather from node tables (4 src-groups of 25088
rows for int16 indices). v2 vs v1:
  - batch-uniform slot layout (W consecutive windows share per-group width
    Ug) so attention + weighted segment-sum run as ~40 large 4D-AP DVE ops
    per batch instead of ~140 small per-window ops;
  - 256B rows (h only); layer-1 a_s is recomputed on the fly from gathered
    h (mult+reduce vs replicated att1_s); pad slots contribute
    exp(lrelu(a_d)) to the softmax denominator, subtracted exactly via a
    host-computed per-(dst,window) pad count;
  - layer-2 rows carry a_s2 in bf16 at col 64 (pad row poisoned to -300);
  - gather DMA waits moved out of the prep critical section (prep b+1
    overlaps DMA b); one idx load + one trigger per batch;
  - phase 0 sharded: each core computes its 12544-row slice of the dense
    h1 table, AllGathered while the per-window a_d pass runs;
  - layer-2 log-softmax ln() deferred to one end pass; single output DMA.
"""

import os
import numpy as np
import ml_dtypes

import concourse.bacc as bacc
import concourse.bass as bass
import concourse.mybir as mybir
import concourse.tile as tile
from concourse.bass_utils import run_bass_kernel_spmd
from concourse.masks import make_identity

F32 = mybir.dt.float32
BF16 = mybir.dt.bfloat16
I16 = mybir.dt.int16
AX = mybir.AxisListType
OP = mybir.AluOpType
ACT = mybir.ActivationFunctionType

N, E = 100000, 1600000
IN, HID, OUT, HEADS = 256, 16, 64, 8
NEG = 0.2
NCORES = 8
NSH = N // NCORES        # 12500
NGRP = 4
GSZ = N // NGRP          # 25000
NP = 25088               # table rows per group
NW = 98                  # windows per core
SH_ROWS = NW * 128       # 12544 = rows per core shard of both tables
PAD1 = GSZ               # group-local pad row, table1 (all-zero row)
PAD2 = NSH               # group-local pad row, table2 (a_s2 = -300)
ROW1 = 256               # bf16 elems per table1 row (512B: a_s f32x8 | h 128)
ROW2 = 128               # bf16 elems per table2 row (256B: h2 64 | a_s2 | pad)
COLS1 = 80               # padded slot columns per layer-1 batch
COLS2 = 120              # padded slot columns per layer-2 batch
# per-batch gather descs = cols*128 must stay under the 16384-desc SWDGE ring
STAGE_ELEMS = COLS1 * ROW1   # >= COLS2 * ROW2, shared stag tile size
WMAX = 7                 # max windows per batch (W*66 <= 512 psum floats)
A_NEG = -300.0


# ---------------------------------------------------------------- host side
def _layout(src, dst):
    core = dst // NSH
    grp = src // GSZ
    cg_all = np.zeros((NCORES, NSH, NGRP), np.int32)
    np.add.at(cg_all, (core, dst % NSH, grp), 1)
    perms = []
    for k in range(NCORES):
        cg = cg_all[k]
        perms.append(np.lexsort((cg[:, 3], cg[:, 2], cg[:, 1], cg[:, 0]))[::-1])
    Lg = np.zeros((NW, NGRP), np.int64)
    for k in range(NCORES):
        cgp = cg_all[k][perms[k]]
        cgp = np.concatenate([cgp, np.zeros((SH_ROWS - NSH, NGRP), np.int32)])
        Lg = np.maximum(Lg, cgp.reshape(NW, 128, NGRP).max(axis=1))
    sig = np.empty(N, np.int64)
    for k in range(NCORES):
        pos = np.empty(NSH, np.int64)
        pos[perms[k]] = np.arange(NSH)
        sig[k * NSH:(k + 1) * NSH] = k * SH_ROWS + pos
    eorder = np.lexsort((grp, dst))
    es, ed, eg, ec = src[eorder], dst[eorder], grp[eorder], core[eorder]
    core_starts = np.searchsorted(ec, np.arange(NCORES + 1))
    cores = [(es[a:b], (ed[a:b] - k * NSH), eg[a:b])
             for k, (a, b) in enumerate(zip(core_starts[:-1], core_starts[1:]))]
    return dict(Lg=Lg, perms=perms, sig=sig, cores=cores)


def _make_batches(Lg, budget):
    batches = []
    w = 0
    while w < NW:
        best = None
        for W in range(1, WMAX + 1):
            if w + W > NW:
                break
            Ug = Lg[w:w + W].max(axis=0)
            cols = W * int(Ug.sum())
            if cols <= budget:
                best = (W, Ug)
        if best is None:
            best = (1, Lg[w:w + 1].max(axis=0))
        W, Ug = best
        Ug = [int(u) for u in Ug]
        batches.append(dict(w0=w, W=W, Ug=Ug, Utot=sum(Ug)))
        w += W
    return batches


def _pack_idx(arr_pj):
    """[128, cols] slot-array of indices -> wrapped idx tile [128, cols*8]."""
    I = arr_pj.T.ravel()                      # I[j*128+p]
    W = I.reshape(-1, 16).T.astype(np.int16)  # [16, len/16]
    return np.tile(W, (8, 1))


def _host_inputs(inputs, lay, batches1, batches2):
    x = np.asarray(inputs["x"], np.float32)
    W1 = np.asarray(inputs["W1"], np.float64)
    att1_s = np.asarray(inputs["att1_s"], np.float64)
    att1_d = np.asarray(inputs["att1_d"], np.float64)
    W2 = np.asarray(inputs["W2"], np.float64)
    att2_s = np.asarray(inputs["att2_s"], np.float64)
    att2_d = np.asarray(inputs["att2_d"], np.float64)
    b1 = np.asarray(inputs["b1"], np.float32)
    b2 = np.asarray(inputs["b2"], np.float32)
    Lg, perms, sig = lay["Lg"], lay["perms"], lay["sig"]

    A_s = np.zeros((HEADS * HID, HEADS))
    A_d = np.zeros((HEADS * HID, HEADS))
    for h in range(HEADS):
        A_s[h * HID:(h + 1) * HID, h] = att1_s[h]
        A_d[h * HID:(h + 1) * HID, h] = att1_d[h]
    w1r = np.concatenate([W1, W1 @ A_s, W1 @ A_d], axis=1)           # [256,144]
    w2r = np.concatenate([W2, W2 @ att2_s.T, W2 @ att2_d.T], axis=1)  # [128,66]
    w1r_bf = np.ascontiguousarray(w1r.astype(ml_dtypes.bfloat16))
    w2r_bf = np.ascontiguousarray(w2r.astype(ml_dtypes.bfloat16))

    xT = np.zeros((IN, NGRP * NP), np.float32)
    for g in range(NGRP):
        xT[:, g * NP:g * NP + GSZ] = x[g * GSZ:(g + 1) * GSZ].T
    xT_bf = xT.astype(ml_dtypes.bfloat16)

    cst_base = np.zeros((128, 290), np.float32)
    cst_base[:, 0:128] = b1[None, :]
    cst_base[:, 128:192] = b2[None, :]

    per_core = []
    for k in range(NCORES):
        es, edl, eg = lay["cores"][k]
        pos = np.empty(NSH, np.int64)
        pos[perms[k]] = np.arange(NSH)
        o = np.lexsort((eg, pos[edl]))
        es_o, eg_o, pos_o = es[o], eg[o], pos[edl][o]
        w_o, p_o = pos_o // 128, pos_o % 128
        key = pos_o * NGRP + eg_o
        slot = np.arange(len(o)) - np.searchsorted(key, key)
        deg = np.zeros((128, NW), np.int32)
        np.add.at(deg, (p_o, w_o), 1)
        npad = np.zeros((128, NW), np.float32)
        secs = []
        for li, batches in enumerate((batches1, batches2)):
            vals = es_o % GSZ if li == 0 else sig[es_o] % NP
            padv = PAD1 if li == 0 else PAD2
            for b in batches:
                w0, W, Ug = b["w0"], b["W"], b["Ug"]
                if li == 0:
                    npad[:, w0:w0 + W] = b["Utot"] - deg[:, w0:w0 + W]
                inb = (w_o >= w0) & (w_o < w0 + W)
                for g in range(NGRP):
                    if Ug[g] == 0:
                        continue
                    a = np.full((128, W * Ug[g]), padv, np.int64)
                    m = inb & (eg_o == g)
                    a[p_o[m], (w_o[m] - w0) * Ug[g] + slot[m]] = vals[m]
                    secs.append(a)
        idx_blob = np.concatenate([_pack_idx(a) for a in secs], axis=1)
        xtp = np.zeros((IN, SH_ROWS), np.float32)
        xtp[:, :NSH] = x[k * NSH:(k + 1) * NSH].T[:, perms[k]]
        cst = cst_base.copy()
        cst[:, 192:290] = npad
        per_core.append({
            "xts": np.ascontiguousarray(xT_bf[:, k * SH_ROWS:(k + 1) * SH_ROWS]),
            "xtp": np.ascontiguousarray(xtp.astype(ml_dtypes.bfloat16)),
            "w1r": w1r_bf,
            "w2r": w2r_bf,
            "cst": np.ascontiguousarray(cst),
            "idx": np.ascontiguousarray(idx_blob),
        })
    return per_core


# ------------------------------------------------------------- device side
def _build_program(Lg, batches1, batches2):
    nc = bacc.Bacc("TRN2", target_bir_lowering=False, debug=False,
                   num_devices=NCORES)
    IDXF = 8 * (sum(b["W"] * b["Utot"] for b in batches1)
                + sum(b["W"] * b["Utot"] for b in batches2))
    MAXGRP = max(max(b["W"] * max(b["Ug"]) for b in batches1) * HEADS * HID,
                 max(b["W"] * max(b["Ug"]) for b in batches2) * OUT)
    xts = nc.declare_dram_parameter("xts", [256, SH_ROWS], BF16, isOutput=False)
    xtp = nc.declare_dram_parameter("xtp", [256, SH_ROWS], BF16, isOutput=False)
    w1r = nc.declare_dram_parameter("w1r", [256, 144], BF16, isOutput=False)
    w2r = nc.declare_dram_parameter("w2r", [128, 66], BF16, isOutput=False)
    cst = nc.declare_dram_parameter("cst", [128, 290], F32, isOutput=False)
    idxp = nc.declare_dram_parameter("idx", [128, IDXF], I16, isOutput=False)
    outp = nc.declare_dram_parameter("out", [SH_ROWS, OUT], F32, isOutput=True)

    shard1 = nc.dram_tensor("shard1", [SH_ROWS, ROW1], BF16)
    table1 = nc.dram_tensor("table1", [NCORES * SH_ROWS, ROW1], BF16)
    shard2 = nc.dram_tensor("shard2", [SH_ROWS, ROW2], BF16)
    table2 = nc.dram_tensor("table2", [NCORES * SH_ROWS, ROW2], BF16)

    dma_sem = nc.alloc_semaphore("g_dma")
    prep_sem = nc.alloc_semaphore("g_prep")
    cc_sem = nc.alloc_semaphore("cc")
    gn = [0]   # gathers issued
    cn = [0]   # collectives issued

    CH = 14          # windows/tiles per phase-0 chunk
    NCH = NW // CH   # 7

    with tile.TileContext(nc) as tc:
        with (
            tc.tile_pool(name="const", bufs=1) as constp,
            tc.tile_pool(name="psum", bufs=2, space="PSUM") as psump,
        ):
            w1r0_t = constp.tile([128, 144], BF16, tag="w1r0")
            w1r1_t = constp.tile([128, 144], BF16, tag="w1r1")
            w2r_t = constp.tile([128, 66], BF16, tag="w2r")
            cst_t = constp.tile([128, 290], F32, tag="cst")
            ident = constp.tile([128, 128], BF16, tag="ident")
            adwin = constp.tile([128, NW * HEADS], F32, tag="adwin")
            ad2win = constp.tile([128, NW], F32, tag="ad2win")
            npe1 = constp.tile([128, NW * HEADS], F32, tag="npe1")
            shbuf = constp.tile([128, NW * OUT], F32, tag="shbuf")
            sebuf = constp.tile([128, NW], F32, tag="sebuf")
            nc.sync.dma_start(out=w1r0_t[:], in_=w1r[0:128, :])
            nc.sync.dma_start(out=w1r1_t[:], in_=w1r[128:256, :])
            nc.sync.dma_start(out=w2r_t[:], in_=w2r[:])
            nc.sync.dma_start(out=cst_t[:], in_=cst[:])
            make_identity(nc, ident[:])
            b1v = cst_t[:, 0:128]
            b2v = cst_t[:, 128:192]
            npadv = cst_t[:, 192:290]

            # ---------------- phase 0: own slice of dense h1 table ---------
            with (
                tc.tile_pool(name="xt", bufs=2) as xtpool,
                tc.tile_pool(name="dense", bufs=2) as densep,
            ):
                for ch in range(NCH):
                    base = ch * CH * 128
                    xs0 = xtpool.tile([128, CH * 128], BF16, tag="xs0")
                    xs1 = xtpool.tile([128, CH * 128], BF16, tag="xs1")
                    nc.sync.dma_start(out=xs0[:], in_=xts[0:128, base:base + CH * 128])
                    nc.sync.dma_start(out=xs1[:], in_=xts[128:256, base:base + CH * 128])
                    rows = densep.tile([128, CH * 144], BF16, tag="rows")
                    for t in range(CH):
                        ps = psump.tile([128, 136], F32, tag="ps0")
                        nc.tensor.matmul(
                            out=ps[:], lhsT=xs0[:, t * 128:(t + 1) * 128],
                            rhs=w1r0_t[:, 0:136], start=True, stop=False)
                        nc.tensor.matmul(
                            out=ps[:], lhsT=xs1[:, t * 128:(t + 1) * 128],
                            rhs=w1r1_t[:, 0:136], start=False, stop=True)
                        rv = rows[:, t * 144:(t + 1) * 144]
                        nc.scalar.activation(rv[0:128, 0:16].bitcast(F32),
                                             ps[:, 128:136], ACT.Copy, 0.0, 1.0)
                        nc.scalar.activation(rv[0:128, 16:144],
                                             ps[:, 0:128], ACT.Copy, 0.0, 1.0)
                    nc.sync.dma_start(
                        out=shard1[base:base + CH * 128, 0:144]
                            .rearrange("(a p) r -> p a r", p=128),
                        in_=rows[:].rearrange("p (a r) -> p a r", a=CH))

                # AllGather table1 (overlaps with the a_d pass below)
                with tc.tile_critical():
                    nc.gpsimd.collective_compute(
                        "AllGather", OP.bypass,
                        replica_groups=[list(range(NCORES))],
                        ins=[shard1[:]], outs=[table1[:]],
                    ).then_inc(cc_sem, 1)
                    nc.gpsimd.wait_ge(cc_sem, 1)
                cn[0] += 1

                # a_d per window (window-ordered x.T)
                for ch in range(NCH):
                    base = ch * CH * 128
                    xp0 = xtpool.tile([128, CH * 128], BF16, tag="xs0")
                    xp1 = xtpool.tile([128, CH * 128], BF16, tag="xs1")
                    nc.sync.dma_start(out=xp0[:], in_=xtp[0:128, base:base + CH * 128])
                    nc.sync.dma_start(out=xp1[:], in_=xtp[128:256, base:base + CH * 128])
                    psa = psump.tile([128, CH * HEADS], F32, tag="psa")
                    for t in range(CH):
                        nc.tensor.matmul(
                            out=psa[:, t * 8:(t + 1) * 8],
                            lhsT=xp0[:, t * 128:(t + 1) * 128],
                            rhs=w1r0_t[:, 136:144], start=True, stop=False)
                        nc.tensor.matmul(
                            out=psa[:, t * 8:(t + 1) * 8],
                            lhsT=xp1[:, t * 128:(t + 1) * 128],
                            rhs=w1r1_t[:, 136:144], start=False, stop=True)
                    nc.vector.tensor_copy(
                        out=adwin[:, ch * CH * 8:(ch + 1) * CH * 8], in_=psa[:])

            # npe1 = npad * exp(lrelu(adwin))
            nc.vector.tensor_scalar_mul(npe1[:], adwin[:], NEG)
            nc.vector.tensor_tensor(out=npe1[:], in0=adwin[:], in1=npe1[:],
                                    op=OP.max)
            nc.scalar.activation(npe1[:], npe1[:], ACT.Exp, 0.0, 1.0)
            nc.vector.tensor_tensor(
                out=npe1[:].rearrange("p (w h) -> p w h", w=NW),
                in0=npe1[:].rearrange("p (w h) -> p w h", w=NW),
                in1=npadv.rearrange("p (w h) -> p w h", h=1)
                    .to_broadcast([128, NW, HEADS]),
                op=OP.mult)

            # ---------------- edge layers ----------------------------------
            ctx_edge = __import__("contextlib").ExitStack()
            stagp = ctx_edge.enter_context(tc.tile_pool(name="stag", bufs=2))
            idxpool = ctx_edge.enter_context(tc.tile_pool(name="idx", bufs=2))
            workp = ctx_edge.enter_context(tc.tile_pool(name="work", bufs=2))
            scrp = ctx_edge.enter_context(tc.tile_pool(name="scr", bufs=1))
            smallp = ctx_edge.enter_context(tc.tile_pool(name="small", bufs=2))
            idx_off = [0]

            def issue_gathers(layer, b, stag, ixt):
                tabl, row = (table1, ROW1) if layer == 1 else (table2, ROW2)
                W, Ug = b["W"], b["Ug"]
                ng0 = gn[0]
                with tc.tile_critical():
                    goff = 0
                    for g in range(NGRP):
                        if Ug[g] == 0:
                            continue
                        Kg = W * Ug[g]
                        sl3 = stag[:, goff * row:(goff + Kg) * row] \
                            .rearrange("p (k r) -> p k r", r=row)
                        gn[0] += 1
                        nc.gpsimd.dma_gather(
                            out_ap=sl3, in_ap=tabl[g * NP:(g + 1) * NP, :],
                            idxs_ap=ixt[:, goff * 8:(goff + Kg) * 8],
                            num_idxs=128 * Kg, num_idxs_reg=128 * Kg,
                            elem_size=row, single_packet=False,
                            prepare_only=True, sem=dma_sem,
                        ).then_inc(prep_sem, 1)
                        goff += Kg
                    nc.gpsimd.wait_ge(prep_sem, gn[0])
                    nc.gpsimd.trigger_dma(count=gn[0] - ng0)
                return gn[0]

            def await_gather(layer, b, stag, gtarget):
                row = ROW1 if layer == 1 else ROW2
                cols = b["W"] * b["Utot"]
                with tc.tile_critical():
                    nc.gpsimd.wait_ge(dma_sem, 16 * gtarget)
                    v = stag[:, 0:cols * row].rearrange("p (k r) -> p k r", r=row)
                    nc.gpsimd.tensor_copy(out=v[:, :, 0:1], in_=v[:, :, 0:1])

            def load_batch(layer, b):
                cols = b["W"] * b["Utot"]
                ixt = idxpool.tile([128, COLS2 * 8], I16, tag="ix")
                nc.sync.dma_start(out=ixt[:, 0:cols * 8],
                                  in_=idxp[:, idx_off[0]:idx_off[0] + cols * 8])
                idx_off[0] += cols * 8
                stag = stagp.tile([128, STAGE_ELEMS], BF16, tag="st")
                gtarget = issue_gathers(layer, b, stag, ixt)
                return stag, gtarget

            def compute_batch(layer, b, stag):
                w0, W, Ug, Utot = b["w0"], b["W"], b["Ug"], b["Utot"]
                nh = HEADS if layer == 1 else 1
                nch = HID if layer == 1 else OUT
                row = ROW1 if layer == 1 else ROW2
                hoff = 16 if layer == 1 else 0
                wall_t = workp.tile([128, COLS1 * HEADS], F32, tag="wa")
                # --- logits = a_s (in-row) + a_d, group-major into wall -----
                goff = 0
                for g in range(NGRP):
                    if Ug[g] == 0:
                        continue
                    Kg = W * Ug[g]
                    sec4 = stag[:, goff * row:(goff + Kg) * row].rearrange(
                        "p (w l r) -> p w l r", w=W, r=row)
                    if layer == 1:
                        asv = sec4[:, :, :, 0:16].bitcast(F32)
                        adv = adwin[:, w0 * 8:(w0 + W) * 8]
                    else:
                        asv = sec4[:, :, :, 64:65]
                        adv = ad2win[:, w0:w0 + W]
                    wv = wall_t[:, goff * nh:(goff + Kg) * nh]
                    nc.vector.tensor_tensor(
                        out=wv.rearrange("p (w l h) -> p w l h", w=W, h=nh),
                        in0=asv,
                        in1=adv.rearrange("p (w l h) -> p w l h", w=W, l=1)
                            .to_broadcast([128, W, Ug[g], nh]),
                        op=OP.add)
                    goff += Kg
                wall = wall_t[:, 0:Utot * W * nh]
                # --- lrelu + exp --------------------------------------------
                lr = smallp.tile([128, COLS1 * HEADS], F32, tag="lr")
                nc.vector.tensor_scalar_mul(lr[:, 0:Utot * W * nh], wall, NEG)
                nc.vector.tensor_tensor(out=wall, in0=wall,
                                        in1=lr[:, 0:Utot * W * nh], op=OP.max)
                nc.scalar.activation(wall, wall, ACT.Exp, 0.0, 1.0)
                # --- denominator -------------------------------------------
                den = smallp.tile([128, WMAX * HEADS], F32, tag="den")
                dent = smallp.tile([128, WMAX * HEADS], F32, tag="dent")
                goff = 0
                first = True
                for g in range(NGRP):
                    if Ug[g] == 0:
                        continue
                    Kg = W * Ug[g]
                    tgt = den if first else dent
                    nc.vector.tensor_reduce(
                        out=tgt[:, 0:W * nh].rearrange("p (w h) -> p w h", h=nh),
                        in_=wall_t[:, goff * nh:(goff + Kg) * nh].rearrange(
                            "p (w l h) -> p w h l", w=W, h=nh),
                        axis=AX.X, op=OP.add)
                    if not first:
                        nc.vector.tensor_tensor(out=den[:, 0:W * nh],
                                                in0=den[:, 0:W * nh],
                                                in1=dent[:, 0:W * nh], op=OP.add)
                    first = False
                    goff += Kg
                if layer == 1:
                    nc.vector.tensor_tensor(out=den[:, 0:W * nh],
                                            in0=den[:, 0:W * nh],
                                            in1=npe1[:, w0 * 8:(w0 + W) * 8],
                                            op=OP.subtract)
                nc.vector.tensor_scalar_max(den[:, 0:W * nh], den[:, 0:W * nh],
                                            1e-30)
                rec = smallp.tile([128, WMAX * HEADS], F32, tag="rec")
                nc.vector.reciprocal(rec[:, 0:W * nh], den[:, 0:W * nh])
                # --- weighted message sum ----------------------------------
                opre = smallp.tile([128, WMAX * 128], F32, tag="opre")
                opret = smallp.tile([128, WMAX * 128], F32, tag="opret")
                goff = 0
                first = True
                for g in range(NGRP):
                    if Ug[g] == 0:
                        continue
                    Kg = W * Ug[g]
                    hv = stag[:, goff * row:(goff + Kg) * row] \
                        .rearrange("p (k r) -> p k r", r=row)[:, :, hoff:hoff + nh * nch]
                    msg = scrp.tile([128, MAXGRP], BF16, tag="pr")
                    mv = msg[:, 0:Kg * nh * nch].rearrange(
                        "p (k h c) -> p k h c", h=nh, c=nch)
                    nc.vector.tensor_tensor(
                        out=mv,
                        in0=hv.rearrange("p k (h c) -> p k h c", h=nh),
                        in1=wall_t[:, goff * nh:(goff + Kg) * nh].rearrange(
                            "p (k h c) -> p k h c", h=nh, c=1)
                            .to_broadcast([128, Kg, nh, nch]),
                        op=OP.mult)
                    tgt = opre if first else opret
                    nc.vector.tensor_reduce(
                        out=tgt[:, 0:W * nh * nch].rearrange(
                            "p (w e) -> p w e", w=W),
                        in_=msg[:, 0:Kg * nh * nch].rearrange(
                            "p (w l e) -> p w e l", w=W, l=Ug[g]),
                        axis=AX.X, op=OP.add)
                    if not first:
                        nc.vector.tensor_tensor(out=opre[:, 0:W * nh * nch],
                                                in0=opre[:, 0:W * nh * nch],
                                                in1=opret[:, 0:W * nh * nch],
                                                op=OP.add)
                    first = False
                    goff += Kg
                # --- normalize + bias --------------------------------------
                o1 = smallp.tile([128, WMAX * 128], F32, tag="o1")
                nc.vector.tensor_tensor(
                    out=o1[:, 0:W * nh * nch].rearrange(
                        "p (w h c) -> p w h c", h=nh, c=nch),
                    in0=opre[:, 0:W * nh * nch].rearrange(
                        "p (w h c) -> p w h c", h=nh, c=nch),
                    in1=rec[:, 0:W * nh].rearrange(
                        "p (w h c) -> p w h c", h=nh, c=1)
                        .to_broadcast([128, W, nh, nch]),
                    op=OP.mult)
                bv = b1v if layer == 1 else b2v
                nc.vector.tensor_tensor(
                    out=o1[:, 0:W * nh * nch].rearrange(
                        "p (w e) -> p w e", w=W),
                    in0=o1[:, 0:W * nh * nch].rearrange(
                        "p (w e) -> p w e", w=W),
                    in1=bv.rearrange("p (w e) -> p w e", w=1)
                        .to_broadcast([128, W, nh * nch]),
                    op=OP.add)
                if layer == 1:
                    # ELU -> h2 rows -> shard2
                    ne = W * 128
                    tneg = smallp.tile([128, WMAX * 128], F32, tag="tneg")
                    nc.vector.tensor_scalar_min(tneg[:, 0:ne], o1[:, 0:ne], 0.0)
                    nc.scalar.activation(tneg[:, 0:ne], tneg[:, 0:ne],
                                         ACT.Exp, 0.0, 1.0)
                    nc.vector.tensor_scalar_max(o1[:, 0:ne], o1[:, 0:ne], 0.0)
                    nc.vector.tensor_tensor(out=o1[:, 0:ne], in0=o1[:, 0:ne],
                                            in1=tneg[:, 0:ne], op=OP.add)
                    nc.vector.tensor_scalar_add(o1[:, 0:ne], o1[:, 0:ne], -1.0)
                    o1bf = smallp.tile([128, WMAX * 128], BF16, tag="o1bf")
                    nc.vector.tensor_copy(out=o1bf[:, 0:ne], in_=o1[:, 0:ne])
                    ps2 = psump.tile([128, WMAX * 66], F32, tag="ps2")
                    for wi in range(W):
                        pst = psump.tile([128, 128], BF16, tag="pst")
                        nc.tensor.transpose(
                            out=pst[:], in_=o1bf[:, wi * 128:(wi + 1) * 128],
                            identity=ident[:])
                        o1T = smallp.tile([128, 128], BF16, tag="o1T")
                        nc.vector.tensor_copy(out=o1T[:], in_=pst[:])
                        nc.tensor.matmul(out=ps2[:, wi * 66:(wi + 1) * 66],
                                         lhsT=o1T[:], rhs=w2r_t[:],
                                         start=True, stop=True)
                    row2 = smallp.tile([128, WMAX * 65], BF16, tag="row2")
                    nc.vector.tensor_copy(
                        out=row2[:, 0:W * 65].rearrange(
                            "p (w r) -> p w r", w=W),
                        in_=ps2[:, 0:W * 66].rearrange(
                            "p (w r) -> p w r", w=W)[:, :, 0:65])
                    nc.vector.tensor_copy(
                        out=ad2win[:, w0:w0 + W].rearrange(
                            "p (w r) -> p w r", r=1),
                        in_=ps2[:, 0:W * 66].rearrange(
                            "p (w r) -> p w r", w=W)[:, :, 65:66])
                    nc.sync.dma_start(
                        out=shard2[w0 * 128:(w0 + W) * 128, 0:65]
                            .rearrange("(a p) r -> p a r", p=128),
                        in_=row2[:, 0:W * 65].rearrange("p (a r) -> p a r", a=W))
                else:
                    # log-softmax: sh and se into persistent buffers
                    ne = W * OUT
                    mx = smallp.tile([128, WMAX], F32, tag="mx")
                    nc.vector.tensor_reduce(
                        out=mx[:, 0:W].rearrange("p (w e) -> p w e", e=1),
                        in_=o1[:, 0:ne].rearrange("p (w c) -> p w c", w=W),
                        axis=AX.X, op=OP.max)
                    shv = shbuf[:, w0 * OUT:(w0 + W) * OUT]
                    nc.vector.tensor_tensor(
                        out=shv.rearrange("p (w c) -> p w c", w=W),
                        in0=o1[:, 0:ne].rearrange("p (w c) -> p w c", w=W),
                        in1=mx[:, 0:W].rearrange("p (w c) -> p w c", c=1)
                            .to_broadcast([128, W, OUT]),
                        op=OP.subtract)
                    ex = smallp.tile([128, WMAX * OUT], F32, tag="ex")
                    nc.scalar.activation(ex[:, 0:ne], shv, ACT.Exp, 0.0, 1.0)
                    nc.vector.tensor_reduce(
                        out=sebuf[:, w0:w0 + W].rearrange(
                            "p (w e) -> p w e", e=1),
                        in_=ex[:, 0:ne].rearrange("p (w c) -> p w c", w=W),
                        axis=AX.X, op=OP.add)

            def edge_layer(layer):
                nb = int(os.environ.get("GAT_NBATCH", "999"))
                todo = (batches1 if layer == 1 else batches2)[:nb]
                if not todo:
                    return
                pend = [(todo[0], *load_batch(layer, todo[0]))]
                for b in todo[1:]:
                    pend.append((b, *load_batch(layer, b)))
                    bp, st, gt = pend.pop(0)
                    await_gather(layer, bp, st, gt)
                    compute_batch(layer, bp, st)
                bp, st, gt = pend.pop(0)
                await_gather(layer, bp, st, gt)
                compute_batch(layer, bp, st)

            STAGE = int(os.environ.get("GAT_STAGE", "3"))
            if STAGE >= 1:
                edge_layer(1)
            # pad row for table2: a_s2 = -300
            pr2 = constp.tile([1, 1], BF16, tag="pr2")
            nc.vector.memset(pr2[:], A_NEG)
            nc.sync.dma_start(out=shard2[PAD2:PAD2 + 1, 64:65], in_=pr2[0:1, :])
            if STAGE >= 2:
                with tc.tile_critical():
                    nc.gpsimd.collective_compute(
                        "AllGather", OP.bypass,
                        replica_groups=[list(range(NCORES))],
                        ins=[shard2[:]], outs=[table2[:]],
                    ).then_inc(cc_sem, 1)
                    nc.gpsimd.wait_ge(cc_sem, 2)
                cn[0] += 1
            if STAGE >= 3:
                edge_layer(2)
                # final: out = sh - ln(se), one DMA
                nc.scalar.activation(sebuf[:], sebuf[:], ACT.Ln, 0.0, 1.0)
                nc.vector.tensor_tensor(
                    out=shbuf[:].rearrange("p (w c) -> p w c", w=NW),
                    in0=shbuf[:].rearrange("p (w c) -> p w c", w=NW),
                    in1=sebuf[:].rearrange("p (w c) -> p w c", c=1)
                        .to_broadcast([128, NW, OUT]),
                    op=OP.subtract)
                nc.sync.dma_start(
                    out=outp[:].rearrange("(a p) r -> p a r", p=128),
                    in_=shbuf[:].rearrange("p (a r) -> p a r", a=NW))
            else:
                zo = smallp.tile([128, OUT], F32, tag="zo")
                nc.vector.memset(zo[:], 0.0)
                for w in range(NW):
                    nc.sync.dma_start(out=outp[w * 128:(w + 1) * 128, :],
                                      in_=zo[:])
            ctx_edge.close()
    nc.compile()
    return nc


_CACHE = {}


def kernel(**inputs):
    ei = np.asarray(inputs["edge_index"])
    src, dst = ei[0].astype(np.int64), ei[1].astype(np.int64)
    lay = _layout(src, dst)
    batches1 = _make_batches(lay["Lg"], COLS1)
    batches2 = _make_batches(lay["Lg"], COLS2)
    per_core = _host_inputs(inputs, lay, batches1, batches2)
    key = (ei.tobytes()[:64], int(lay["Lg"].sum()))
    if key not in _CACHE:
        _CACHE[key] = _build_program(lay["Lg"], batches1, batches2)
    nc = _CACHE[key]
    res = run_bass_kernel_spmd(nc, per_core, core_ids=list(range(NCORES)))
    out = np.empty((N, OUT), np.float32)
    for k in range(NCORES):
        out[k * NSH + lay["perms"][k]] = res.results[k]["out"][:NSH]
    return out


if __name__ == "__main__":
    d = np.load("/root/problem/_inp_check.npz")
    o = kernel(**{k: d[k] for k in d.files})
    ref = np.load("/root/problem/_ref_check.npy")
    rel = np.linalg.norm(o - ref) / np.linalg.norm(ref)
    err = np.abs(o - ref) / (np.abs(ref) + 1e-5)
    print("fro rel err:", rel, "max elem rel err:", err.max())


# revision 3
# speedup vs baseline: 1.0893x; 1.0071x over previous
"""GAT (2-layer) Trainium2 Bass kernel — 8-core SPMD, v2.

Sharding: dst nodes across 8 cores (12500 each), packed into 98 windows of
128 (one SBUF partition per dst), profile-sorted. Edge rows (256B: h only)
are fetched by gpsimd.dma_gather from node tables (4 src-groups of 25088
rows for int16 indices). v2 vs v1:
  - batch-uniform slot layout (W consecutive windows share per-group width
    Ug) so attention + weighted segment-sum run as ~40 large 4D-AP DVE ops
    per batch instead of ~140 small per-window ops;
  - 256B rows (h only); layer-1 a_s is recomputed on the fly from gathered
    h (mult+reduce vs replicated att1_s); pad slots contribute
    exp(lrelu(a_d)) to the softmax denominator, subtracted exactly via a
    host-computed per-(dst,window) pad count;
  - layer-2 rows carry a_s2 in bf16 at col 64 (pad row poisoned to -300);
  - gather DMA waits moved out of the prep critical section (prep b+1
    overlaps DMA b); one idx load + one trigger per batch;
  - phase 0 sharded: each core computes its 12544-row slice of the dense
    h1 table, AllGathered while the per-window a_d pass runs;
  - layer-2 log-softmax ln() deferred to one end pass; single output DMA.
"""

import os
import numpy as np
import ml_dtypes

import concourse.bacc as bacc
import concourse.bass as bass
import concourse.mybir as mybir
import concourse.tile as tile
from concourse.bass_utils import run_bass_kernel_spmd
from concourse.masks import make_identity

F32 = mybir.dt.float32
BF16 = mybir.dt.bfloat16
I16 = mybir.dt.int16
AX = mybir.AxisListType
OP = mybir.AluOpType
ACT = mybir.ActivationFunctionType

N, E = 100000, 1600000
IN, HID, OUT, HEADS = 256, 16, 64, 8
NEG = 0.2
NCORES = 8
NSH = N // NCORES        # 12500
NGRP = 4
GSZ = N // NGRP          # 25000
NP = 25088               # table rows per group
NW = 98                  # windows per core
SH_ROWS = NW * 128       # 12544 = rows per core shard of both tables
PAD1 = GSZ               # group-local pad row, table1 (all-zero row)
PAD2 = NSH               # group-local pad row, table2 (a_s2 = -300)
ROW1 = 256               # bf16 elems per table1 row (512B: a_s f32x8 | h 128)
ROW2 = 128               # bf16 elems per table2 row (256B: h2 64 | a_s2 | pad)
COLS1 = 96               # padded slot columns per layer-1 batch
COLS2 = 120              # padded slot columns per layer-2 batch
# per-batch gather descs = cols*128 must stay under the 16384-desc SWDGE ring
STAGE_ELEMS = COLS1 * ROW1   # >= COLS2 * ROW2, shared stag tile size
WMAX = 7                 # max windows per batch (W*66 <= 512 psum floats)
A_NEG = -300.0


# ---------------------------------------------------------------- host side
def _layout(src, dst):
    core = dst // NSH
    grp = src // GSZ
    cg_all = np.zeros((NCORES, NSH, NGRP), np.int32)
    np.add.at(cg_all, (core, dst % NSH, grp), 1)
    perms = []
    for k in range(NCORES):
        cg = cg_all[k]
        perms.append(np.lexsort((cg[:, 3], cg[:, 2], cg[:, 1], cg[:, 0]))[::-1])
    Lg = np.zeros((NW, NGRP), np.int64)
    for k in range(NCORES):
        cgp = cg_all[k][perms[k]]
        cgp = np.concatenate([cgp, np.zeros((SH_ROWS - NSH, NGRP), np.int32)])
        Lg = np.maximum(Lg, cgp.reshape(NW, 128, NGRP).max(axis=1))
    sig = np.empty(N, np.int64)
    for k in range(NCORES):
        pos = np.empty(NSH, np.int64)
        pos[perms[k]] = np.arange(NSH)
        sig[k * NSH:(k + 1) * NSH] = k * SH_ROWS + pos
    eorder = np.lexsort((grp, dst))
    es, ed, eg, ec = src[eorder], dst[eorder], grp[eorder], core[eorder]
    core_starts = np.searchsorted(ec, np.arange(NCORES + 1))
    cores = [(es[a:b], (ed[a:b] - k * NSH), eg[a:b])
             for k, (a, b) in enumerate(zip(core_starts[:-1], core_starts[1:]))]
    return dict(Lg=Lg, perms=perms, sig=sig, cores=cores)


def _make_batches(Lg, budget):
    batches = []
    w = 0
    while w < NW:
        best = None
        for W in range(1, WMAX + 1):
            if w + W > NW:
                break
            Ug = Lg[w:w + W].max(axis=0)
            cols = W * int(Ug.sum())
            if cols <= budget:
                best = (W, Ug)
        if best is None:
            best = (1, Lg[w:w + 1].max(axis=0))
        W, Ug = best
        Ug = [int(u) for u in Ug]
        batches.append(dict(w0=w, W=W, Ug=Ug, Utot=sum(Ug)))
        w += W
    return batches


def _pack_idx(arr_pj):
    """[128, cols] slot-array of indices -> wrapped idx tile [128, cols*8]."""
    I = arr_pj.T.ravel()                      # I[j*128+p]
    W = I.reshape(-1, 16).T.astype(np.int16)  # [16, len/16]
    return np.tile(W, (8, 1))


def _host_inputs(inputs, lay, batches1, batches2):
    x = np.asarray(inputs["x"], np.float32)
    W1 = np.asarray(inputs["W1"], np.float64)
    att1_s = np.asarray(inputs["att1_s"], np.float64)
    att1_d = np.asarray(inputs["att1_d"], np.float64)
    W2 = np.asarray(inputs["W2"], np.float64)
    att2_s = np.asarray(inputs["att2_s"], np.float64)
    att2_d = np.asarray(inputs["att2_d"], np.float64)
    b1 = np.asarray(inputs["b1"], np.float32)
    b2 = np.asarray(inputs["b2"], np.float32)
    Lg, perms, sig = lay["Lg"], lay["perms"], lay["sig"]

    A_s = np.zeros((HEADS * HID, HEADS))
    A_d = np.zeros((HEADS * HID, HEADS))
    for h in range(HEADS):
        A_s[h * HID:(h + 1) * HID, h] = att1_s[h]
        A_d[h * HID:(h + 1) * HID, h] = att1_d[h]
    w1r = np.concatenate([W1, W1 @ A_s, W1 @ A_d], axis=1)           # [256,144]
    w2r = np.concatenate([W2, W2 @ att2_s.T, W2 @ att2_d.T], axis=1)  # [128,66]
    w1r_bf = np.ascontiguousarray(w1r.astype(ml_dtypes.bfloat16))
    w2r_bf = np.ascontiguousarray(w2r.astype(ml_dtypes.bfloat16))

    xT = np.zeros((IN, NGRP * NP), np.float32)
    for g in range(NGRP):
        xT[:, g * NP:g * NP + GSZ] = x[g * GSZ:(g + 1) * GSZ].T
    xT_bf = xT.astype(ml_dtypes.bfloat16)

    cst_base = np.zeros((128, 290), np.float32)
    cst_base[:, 0:128] = b1[None, :]
    cst_base[:, 128:192] = b2[None, :]

    per_core = []
    for k in range(NCORES):
        es, edl, eg = lay["cores"][k]
        pos = np.empty(NSH, np.int64)
        pos[perms[k]] = np.arange(NSH)
        o = np.lexsort((eg, pos[edl]))
        es_o, eg_o, pos_o = es[o], eg[o], pos[edl][o]
        w_o, p_o = pos_o // 128, pos_o % 128
        key = pos_o * NGRP + eg_o
        slot = np.arange(len(o)) - np.searchsorted(key, key)
        deg = np.zeros((128, NW), np.int32)
        np.add.at(deg, (p_o, w_o), 1)
        npad = np.zeros((128, NW), np.float32)
        secs = []
        for li, batches in enumerate((batches1, batches2)):
            vals = es_o % GSZ if li == 0 else sig[es_o] % NP
            padv = PAD1 if li == 0 else PAD2
            for b in batches:
                w0, W, Ug = b["w0"], b["W"], b["Ug"]
                if li == 0:
                    npad[:, w0:w0 + W] = b["Utot"] - deg[:, w0:w0 + W]
                inb = (w_o >= w0) & (w_o < w0 + W)
                for g in range(NGRP):
                    if Ug[g] == 0:
                        continue
                    a = np.full((128, W * Ug[g]), padv, np.int64)
                    m = inb & (eg_o == g)
                    a[p_o[m], (w_o[m] - w0) * Ug[g] + slot[m]] = vals[m]
                    secs.append(a)
        idx_blob = np.concatenate([_pack_idx(a) for a in secs], axis=1)
        xtp = np.zeros((IN, SH_ROWS), np.float32)
        xtp[:, :NSH] = x[k * NSH:(k + 1) * NSH].T[:, perms[k]]
        cst = cst_base.copy()
        cst[:, 192:290] = npad
        per_core.append({
            "xts": np.ascontiguousarray(xT_bf[:, k * SH_ROWS:(k + 1) * SH_ROWS]),
            "xtp": np.ascontiguousarray(xtp.astype(ml_dtypes.bfloat16)),
            "w1r": w1r_bf,
            "w2r": w2r_bf,
            "cst": np.ascontiguousarray(cst),
            "idx": np.ascontiguousarray(idx_blob),
        })
    return per_core


# ------------------------------------------------------------- device side
def _build_program(Lg, batches1, batches2):
    nc = bacc.Bacc("TRN2", target_bir_lowering=False, debug=False,
                   num_devices=NCORES)
    IDXF = 8 * (sum(b["W"] * b["Utot"] for b in batches1)
                + sum(b["W"] * b["Utot"] for b in batches2))
    MAXGRP = max(max(b["W"] * max(b["Ug"]) for b in batches1) * HEADS * HID,
                 max(b["W"] * max(b["Ug"]) for b in batches2) * OUT)
    xts = nc.declare_dram_parameter("xts", [256, SH_ROWS], BF16, isOutput=False)
    xtp = nc.declare_dram_parameter("xtp", [256, SH_ROWS], BF16, isOutput=False)
    w1r = nc.declare_dram_parameter("w1r", [256, 144], BF16, isOutput=False)
    w2r = nc.declare_dram_parameter("w2r", [128, 66], BF16, isOutput=False)
    cst = nc.declare_dram_parameter("cst", [128, 290], F32, isOutput=False)
    idxp = nc.declare_dram_parameter("idx", [128, IDXF], I16, isOutput=False)
    outp = nc.declare_dram_parameter("out", [SH_ROWS, OUT], F32, isOutput=True)

    shard1 = nc.dram_tensor("shard1", [SH_ROWS, ROW1], BF16)
    table1 = nc.dram_tensor("table1", [NCORES * SH_ROWS, ROW1], BF16)
    shard2 = nc.dram_tensor("shard2", [SH_ROWS, ROW2], BF16)
    table2 = nc.dram_tensor("table2", [NCORES * SH_ROWS, ROW2], BF16)

    dma_sem = nc.alloc_semaphore("g_dma")
    prep_sem = nc.alloc_semaphore("g_prep")
    cc_sem = nc.alloc_semaphore("cc")
    gn = [0]   # gathers issued
    cn = [0]   # collectives issued

    CH = 14          # windows/tiles per phase-0 chunk
    NCH = NW // CH   # 7

    with tile.TileContext(nc) as tc:
        with (
            tc.tile_pool(name="const", bufs=1) as constp,
            tc.tile_pool(name="psum", bufs=2, space="PSUM") as psump,
        ):
            w1r0_t = constp.tile([128, 144], BF16, tag="w1r0")
            w1r1_t = constp.tile([128, 144], BF16, tag="w1r1")
            w2r_t = constp.tile([128, 66], BF16, tag="w2r")
            cst_t = constp.tile([128, 290], F32, tag="cst")
            ident = constp.tile([128, 128], BF16, tag="ident")
            adwin = constp.tile([128, NW * HEADS], F32, tag="adwin")
            ad2win = constp.tile([128, NW], F32, tag="ad2win")
            npe1 = constp.tile([128, NW * HEADS], F32, tag="npe1")
            shbuf = constp.tile([128, NW * OUT], F32, tag="shbuf")
            sebuf = constp.tile([128, NW], F32, tag="sebuf")
            nc.sync.dma_start(out=w1r0_t[:], in_=w1r[0:128, :])
            nc.sync.dma_start(out=w1r1_t[:], in_=w1r[128:256, :])
            nc.sync.dma_start(out=w2r_t[:], in_=w2r[:])
            nc.sync.dma_start(out=cst_t[:], in_=cst[:])
            make_identity(nc, ident[:])
            b1v = cst_t[:, 0:128]
            b2v = cst_t[:, 128:192]
            npadv = cst_t[:, 192:290]

            # ---------------- phase 0: own slice of dense h1 table ---------
            with (
                tc.tile_pool(name="xt", bufs=2) as xtpool,
                tc.tile_pool(name="dense", bufs=2) as densep,
            ):
                for ch in range(NCH):
                    base = ch * CH * 128
                    xs0 = xtpool.tile([128, CH * 128], BF16, tag="xs0")
                    xs1 = xtpool.tile([128, CH * 128], BF16, tag="xs1")
                    nc.sync.dma_start(out=xs0[:], in_=xts[0:128, base:base + CH * 128])
                    nc.sync.dma_start(out=xs1[:], in_=xts[128:256, base:base + CH * 128])
                    rows = densep.tile([128, CH * 144], BF16, tag="rows")
                    for t in range(CH):
                        ps = psump.tile([128, 136], F32, tag="ps0")
                        nc.tensor.matmul(
                            out=ps[:], lhsT=xs0[:, t * 128:(t + 1) * 128],
                            rhs=w1r0_t[:, 0:136], start=True, stop=False)
                        nc.tensor.matmul(
                            out=ps[:], lhsT=xs1[:, t * 128:(t + 1) * 128],
                            rhs=w1r1_t[:, 0:136], start=False, stop=True)
                        rv = rows[:, t * 144:(t + 1) * 144]
                        nc.scalar.activation(rv[0:128, 0:16].bitcast(F32),
                                             ps[:, 128:136], ACT.Copy, 0.0, 1.0)
                        nc.scalar.activation(rv[0:128, 16:144],
                                             ps[:, 0:128], ACT.Copy, 0.0, 1.0)
                    nc.sync.dma_start(
                        out=shard1[base:base + CH * 128, 0:144]
                            .rearrange("(a p) r -> p a r", p=128),
                        in_=rows[:].rearrange("p (a r) -> p a r", a=CH))

                # AllGather table1 (overlaps with the a_d pass below)
                with tc.tile_critical():
                    nc.gpsimd.collective_compute(
                        "AllGather", OP.bypass,
                        replica_groups=[list(range(NCORES))],
                        ins=[shard1[:]], outs=[table1[:]],
                    ).then_inc(cc_sem, 1)
                    nc.gpsimd.wait_ge(cc_sem, 1)
                cn[0] += 1

                # a_d per window (window-ordered x.T)
                for ch in range(NCH):
                    base = ch * CH * 128
                    xp0 = xtpool.tile([128, CH * 128], BF16, tag="xs0")
                    xp1 = xtpool.tile([128, CH * 128], BF16, tag="xs1")
                    nc.sync.dma_start(out=xp0[:], in_=xtp[0:128, base:base + CH * 128])
                    nc.sync.dma_start(out=xp1[:], in_=xtp[128:256, base:base + CH * 128])
                    psa = psump.tile([128, CH * HEADS], F32, tag="psa")
                    for t in range(CH):
                        nc.tensor.matmul(
                            out=psa[:, t * 8:(t + 1) * 8],
                            lhsT=xp0[:, t * 128:(t + 1) * 128],
                            rhs=w1r0_t[:, 136:144], start=True, stop=False)
                        nc.tensor.matmul(
                            out=psa[:, t * 8:(t + 1) * 8],
                            lhsT=xp1[:, t * 128:(t + 1) * 128],
                            rhs=w1r1_t[:, 136:144], start=False, stop=True)
                    nc.vector.tensor_copy(
                        out=adwin[:, ch * CH * 8:(ch + 1) * CH * 8], in_=psa[:])

            # npe1 = npad * exp(lrelu(adwin))
            nc.vector.tensor_scalar_mul(npe1[:], adwin[:], NEG)
            nc.vector.tensor_tensor(out=npe1[:], in0=adwin[:], in1=npe1[:],
                                    op=OP.max)
            nc.scalar.activation(npe1[:], npe1[:], ACT.Exp, 0.0, 1.0)
            nc.vector.tensor_tensor(
                out=npe1[:].rearrange("p (w h) -> p w h", w=NW),
                in0=npe1[:].rearrange("p (w h) -> p w h", w=NW),
                in1=npadv.rearrange("p (w h) -> p w h", h=1)
                    .to_broadcast([128, NW, HEADS]),
                op=OP.mult)

            # ---------------- edge layers ----------------------------------
            ctx_edge = __import__("contextlib").ExitStack()
            stagp = ctx_edge.enter_context(tc.tile_pool(name="stag", bufs=2))
            idxpool = ctx_edge.enter_context(tc.tile_pool(name="idx", bufs=2))
            workp = ctx_edge.enter_context(tc.tile_pool(name="work", bufs=2))
            scrp = ctx_edge.enter_context(tc.tile_pool(name="scr", bufs=1))
            smallp = ctx_edge.enter_context(tc.tile_pool(name="small", bufs=2))
            idx_off = [0]

            def issue_gathers(layer, b, stag, ixt):
                tabl, row = (table1, ROW1) if layer == 1 else (table2, ROW2)
                W, Ug = b["W"], b["Ug"]
                ng0 = gn[0]
                with tc.tile_critical():
                    goff = 0
                    for g in range(NGRP):
                        if Ug[g] == 0:
                            continue
                        Kg = W * Ug[g]
                        sl3 = stag[:, goff * row:(goff + Kg) * row] \
                            .rearrange("p (k r) -> p k r", r=row)
                        gn[0] += 1
                        nc.gpsimd.dma_gather(
                            out_ap=sl3, in_ap=tabl[g * NP:(g + 1) * NP, :],
                            idxs_ap=ixt[:, goff * 8:(goff + Kg) * 8],
                            num_idxs=128 * Kg, num_idxs_reg=128 * Kg,
                            elem_size=row, single_packet=False,
                            prepare_only=True, sem=dma_sem,
                        ).then_inc(prep_sem, 1)
                        goff += Kg
                    nc.gpsimd.wait_ge(prep_sem, gn[0])
                    nc.gpsimd.trigger_dma(count=gn[0] - ng0)
                return gn[0]

            def await_gather(layer, b, stag, gtarget):
                row = ROW1 if layer == 1 else ROW2
                cols = b["W"] * b["Utot"]
                with tc.tile_critical():
                    nc.gpsimd.wait_ge(dma_sem, 16 * gtarget)
                    v = stag[:, 0:cols * row].rearrange("p (k r) -> p k r", r=row)
                    nc.gpsimd.tensor_copy(out=v[:, :, 0:1], in_=v[:, :, 0:1])

            def load_batch(layer, b):
                cols = b["W"] * b["Utot"]
                ixt = idxpool.tile([128, COLS2 * 8], I16, tag="ix")
                nc.sync.dma_start(out=ixt[:, 0:cols * 8],
                                  in_=idxp[:, idx_off[0]:idx_off[0] + cols * 8])
                idx_off[0] += cols * 8
                stag = stagp.tile([128, STAGE_ELEMS], BF16, tag="st")
                gtarget = issue_gathers(layer, b, stag, ixt)
                return stag, gtarget

            def compute_batch(layer, b, stag):
                w0, W, Ug, Utot = b["w0"], b["W"], b["Ug"], b["Utot"]
                nh = HEADS if layer == 1 else 1
                nch = HID if layer == 1 else OUT
                row = ROW1 if layer == 1 else ROW2
                hoff = 16 if layer == 1 else 0
                wall_t = workp.tile([128, COLS1 * HEADS], F32, tag="wa")
                # --- logits = a_s (in-row) + a_d, group-major into wall -----
                goff = 0
                for g in range(NGRP):
                    if Ug[g] == 0:
                        continue
                    Kg = W * Ug[g]
                    sec4 = stag[:, goff * row:(goff + Kg) * row].rearrange(
                        "p (w l r) -> p w l r", w=W, r=row)
                    if layer == 1:
                        asv = sec4[:, :, :, 0:16].bitcast(F32)
                        adv = adwin[:, w0 * 8:(w0 + W) * 8]
                    else:
                        asv = sec4[:, :, :, 64:65]
                        adv = ad2win[:, w0:w0 + W]
                    wv = wall_t[:, goff * nh:(goff + Kg) * nh]
                    nc.vector.tensor_tensor(
                        out=wv.rearrange("p (w l h) -> p w l h", w=W, h=nh),
                        in0=asv,
                        in1=adv.rearrange("p (w l h) -> p w l h", w=W, l=1)
                            .to_broadcast([128, W, Ug[g], nh]),
                        op=OP.add)
                    goff += Kg
                wall = wall_t[:, 0:Utot * W * nh]
                # --- lrelu + exp --------------------------------------------
                lr = smallp.tile([128, COLS1 * HEADS], F32, tag="lr")
                nc.vector.tensor_scalar_mul(lr[:, 0:Utot * W * nh], wall, NEG)
                nc.vector.tensor_tensor(out=wall, in0=wall,
                                        in1=lr[:, 0:Utot * W * nh], op=OP.max)
                nc.scalar.activation(wall, wall, ACT.Exp, 0.0, 1.0)
                # --- denominator -------------------------------------------
                den = smallp.tile([128, WMAX * HEADS], F32, tag="den")
                dent = smallp.tile([128, WMAX * HEADS], F32, tag="dent")
                goff = 0
                first = True
                for g in range(NGRP):
                    if Ug[g] == 0:
                        continue
                    Kg = W * Ug[g]
                    tgt = den if first else dent
                    nc.vector.tensor_reduce(
                        out=tgt[:, 0:W * nh].rearrange("p (w h) -> p w h", h=nh),
                        in_=wall_t[:, goff * nh:(goff + Kg) * nh].rearrange(
                            "p (w l h) -> p w h l", w=W, h=nh),
                        axis=AX.X, op=OP.add)
                    if not first:
                        nc.vector.tensor_tensor(out=den[:, 0:W * nh],
                                                in0=den[:, 0:W * nh],
                                                in1=dent[:, 0:W * nh], op=OP.add)
                    first = False
                    goff += Kg
                if layer == 1:
                    nc.vector.tensor_tensor(out=den[:, 0:W * nh],
                                            in0=den[:, 0:W * nh],
                                            in1=npe1[:, w0 * 8:(w0 + W) * 8],
                                            op=OP.subtract)
                nc.vector.tensor_scalar_max(den[:, 0:W * nh], den[:, 0:W * nh],
                                            1e-30)
                rec = smallp.tile([128, WMAX * HEADS], F32, tag="rec")
                nc.vector.reciprocal(rec[:, 0:W * nh], den[:, 0:W * nh])
                # --- weighted message sum ----------------------------------
                opre = smallp.tile([128, WMAX * 128], F32, tag="opre")
                opret = smallp.tile([128, WMAX * 128], F32, tag="opret")
                goff = 0
                first = True
                for g in range(NGRP):
                    if Ug[g] == 0:
                        continue
                    Kg = W * Ug[g]
                    hv = stag[:, goff * row:(goff + Kg) * row] \
                        .rearrange("p (k r) -> p k r", r=row)[:, :, hoff:hoff + nh * nch]
                    msg = scrp.tile([128, MAXGRP], BF16, tag="pr")
                    mv = msg[:, 0:Kg * nh * nch].rearrange(
                        "p (k h c) -> p k h c", h=nh, c=nch)
                    nc.vector.tensor_tensor(
                        out=mv,
                        in0=hv.rearrange("p k (h c) -> p k h c", h=nh),
                        in1=wall_t[:, goff * nh:(goff + Kg) * nh].rearrange(
                            "p (k h c) -> p k h c", h=nh, c=1)
                            .to_broadcast([128, Kg, nh, nch]),
                        op=OP.mult)
                    # in-place tree reduction over slots l: contiguous
                    # innermost (e) keeps the DVE 16-bit fast path; pad
                    # slots contribute exact zeros.
                    E_ = nh * nch
                    L = Ug[g]
                    while L > 1:
                        h_ = L // 2
                        v3 = msg[:, 0:Kg * E_].rearrange(
                            "p (w l e) -> p w l e", w=W, l=Ug[g])
                        nc.vector.tensor_tensor(
                            out=v3[:, :, 0:h_, :], in0=v3[:, :, 0:h_, :],
                            in1=v3[:, :, L - h_:L, :], op=OP.add)
                        L -= h_
                    tgt = opre if first else opret
                    nc.vector.tensor_copy(
                        out=tgt[:, 0:W * E_].rearrange("p (w e) -> p w e", w=W),
                        in_=msg[:, 0:Kg * E_].rearrange(
                            "p (w l e) -> p w l e", w=W, l=Ug[g])[:, :, 0:1, :]
                            .rearrange("p w l e -> p w (l e)"))
                    if not first:
                        nc.vector.tensor_tensor(out=opre[:, 0:W * nh * nch],
                                                in0=opre[:, 0:W * nh * nch],
                                                in1=opret[:, 0:W * nh * nch],
                                                op=OP.add)
                    first = False
                    goff += Kg
                # --- normalize + bias --------------------------------------
                o1 = smallp.tile([128, WMAX * 128], F32, tag="o1")
                nc.vector.tensor_tensor(
                    out=o1[:, 0:W * nh * nch].rearrange(
                        "p (w h c) -> p w h c", h=nh, c=nch),
                    in0=opre[:, 0:W * nh * nch].rearrange(
                        "p (w h c) -> p w h c", h=nh, c=nch),
                    in1=rec[:, 0:W * nh].rearrange(
                        "p (w h c) -> p w h c", h=nh, c=1)
                        .to_broadcast([128, W, nh, nch]),
                    op=OP.mult)
                bv = b1v if layer == 1 else b2v
                nc.vector.tensor_tensor(
                    out=o1[:, 0:W * nh * nch].rearrange(
                        "p (w e) -> p w e", w=W),
                    in0=o1[:, 0:W * nh * nch].rearrange(
                        "p (w e) -> p w e", w=W),
                    in1=bv.rearrange("p (w e) -> p w e", w=1)
                        .to_broadcast([128, W, nh * nch]),
                    op=OP.add)
                if layer == 1:
                    # ELU -> h2 rows -> shard2
                    ne = W * 128
                    tneg = smallp.tile([128, WMAX * 128], F32, tag="tneg")
                    nc.vector.tensor_scalar_min(tneg[:, 0:ne], o1[:, 0:ne], 0.0)
                    nc.scalar.activation(tneg[:, 0:ne], tneg[:, 0:ne],
                                         ACT.Exp, 0.0, 1.0)
                    nc.vector.tensor_scalar_max(o1[:, 0:ne], o1[:, 0:ne], 0.0)
                    nc.vector.tensor_tensor(out=o1[:, 0:ne], in0=o1[:, 0:ne],
                                            in1=tneg[:, 0:ne], op=OP.add)
                    nc.vector.tensor_scalar_add(o1[:, 0:ne], o1[:, 0:ne], -1.0)
                    o1bf = smallp.tile([128, WMAX * 128], BF16, tag="o1bf")
                    nc.vector.tensor_copy(out=o1bf[:, 0:ne], in_=o1[:, 0:ne])
                    ps2 = psump.tile([128, WMAX * 66], F32, tag="ps2")
                    for wi in range(W):
                        pst = psump.tile([128, 128], BF16, tag="pst")
                        nc.tensor.transpose(
                            out=pst[:], in_=o1bf[:, wi * 128:(wi + 1) * 128],
                            identity=ident[:])
                        o1T = smallp.tile([128, 128], BF16, tag="o1T")
                        nc.vector.tensor_copy(out=o1T[:], in_=pst[:])
                        nc.tensor.matmul(out=ps2[:, wi * 66:(wi + 1) * 66],
                                         lhsT=o1T[:], rhs=w2r_t[:],
                                         start=True, stop=True)
                    row2 = smallp.tile([128, WMAX * 65], BF16, tag="row2")
                    nc.vector.tensor_copy(
                        out=row2[:, 0:W * 65].rearrange(
                            "p (w r) -> p w r", w=W),
                        in_=ps2[:, 0:W * 66].rearrange(
                            "p (w r) -> p w r", w=W)[:, :, 0:65])
                    nc.vector.tensor_copy(
                        out=ad2win[:, w0:w0 + W].rearrange(
                            "p (w r) -> p w r", r=1),
                        in_=ps2[:, 0:W * 66].rearrange(
                            "p (w r) -> p w r", w=W)[:, :, 65:66])
                    nc.sync.dma_start(
                        out=shard2[w0 * 128:(w0 + W) * 128, 0:65]
                            .rearrange("(a p) r -> p a r", p=128),
                        in_=row2[:, 0:W * 65].rearrange("p (a r) -> p a r", a=W))
                else:
                    # log-softmax: sh and se into persistent buffers
                    ne = W * OUT
                    mx = smallp.tile([128, WMAX], F32, tag="mx")
                    nc.vector.tensor_reduce(
                        out=mx[:, 0:W].rearrange("p (w e) -> p w e", e=1),
                        in_=o1[:, 0:ne].rearrange("p (w c) -> p w c", w=W),
                        axis=AX.X, op=OP.max)
                    shv = shbuf[:, w0 * OUT:(w0 + W) * OUT]
                    nc.vector.tensor_tensor(
                        out=shv.rearrange("p (w c) -> p w c", w=W),
                        in0=o1[:, 0:ne].rearrange("p (w c) -> p w c", w=W),
                        in1=mx[:, 0:W].rearrange("p (w c) -> p w c", c=1)
                            .to_broadcast([128, W, OUT]),
                        op=OP.subtract)
                    ex = smallp.tile([128, WMAX * OUT], F32, tag="ex")
                    nc.scalar.activation(ex[:, 0:ne], shv, ACT.Exp, 0.0, 1.0)
                    nc.vector.tensor_reduce(
                        out=sebuf[:, w0:w0 + W].rearrange(
                            "p (w e) -> p w e", e=1),
                        in_=ex[:, 0:ne].rearrange("p (w c) -> p w c", w=W),
                        axis=AX.X, op=OP.add)

            def edge_layer(layer):
                nb = int(os.environ.get("GAT_NBATCH", "999"))
                todo = (batches1 if layer == 1 else batches2)[:nb]
                if not todo:
                    return
                pend = [(todo[0], *load_batch(layer, todo[0]))]
                for b in todo[1:]:
                    pend.append((b, *load_batch(layer, b)))
                    bp, st, gt = pend.pop(0)
                    await_gather(layer, bp, st, gt)
                    compute_batch(layer, bp, st)
                bp, st, gt = pend.pop(0)
                await_gather(layer, bp, st, gt)
                compute_batch(layer, bp, st)

            STAGE = int(os.environ.get("GAT_STAGE", "3"))
            if STAGE >= 1:
                edge_layer(1)
            # pad row for table2: a_s2 = -300
            pr2 = constp.tile([1, 1], BF16, tag="pr2")
            nc.vector.memset(pr2[:], A_NEG)
            nc.sync.dma_start(out=shard2[PAD2:PAD2 + 1, 64:65], in_=pr2[0:1, :])
            if STAGE >= 2:
                with tc.tile_critical():
                    nc.gpsimd.collective_compute(
                        "AllGather", OP.bypass,
                        replica_groups=[list(range(NCORES))],
                        ins=[shard2[:]], outs=[table2[:]],
                    ).then_inc(cc_sem, 1)
                    nc.gpsimd.wait_ge(cc_sem, 2)
                cn[0] += 1
            if STAGE >= 3:
                edge_layer(2)
                # final: out = sh - ln(se), one DMA
                nc.scalar.activation(sebuf[:], sebuf[:], ACT.Ln, 0.0, 1.0)
                nc.vector.tensor_tensor(
                    out=shbuf[:].rearrange("p (w c) -> p w c", w=NW),
                    in0=shbuf[:].rearrange("p (w c) -> p w c", w=NW),
                    in1=sebuf[:].rearrange("p (w c) -> p w c", c=1)
                        .to_broadcast([128, NW, OUT]),
                    op=OP.subtract)
                nc.sync.dma_start(
                    out=outp[:].rearrange("(a p) r -> p a r", p=128),
                    in_=shbuf[:].rearrange("p (a r) -> p a r", a=NW))
            else:
                zo = smallp.tile([128, OUT], F32, tag="zo")
                nc.vector.memset(zo[:], 0.0)
                for w in range(NW):
                    nc.sync.dma_start(out=outp[w * 128:(w + 1) * 128, :],
                                      in_=zo[:])
            ctx_edge.close()
    nc.compile()
    return nc


_CACHE = {}


def kernel(**inputs):
    ei = np.asarray(inputs["edge_index"])
    src, dst = ei[0].astype(np.int64), ei[1].astype(np.int64)
    lay = _layout(src, dst)
    batches1 = _make_batches(lay["Lg"], COLS1)
    batches2 = _make_batches(lay["Lg"], COLS2)
    per_core = _host_inputs(inputs, lay, batches1, batches2)
    key = (ei.tobytes()[:64], int(lay["Lg"].sum()))
    if key not in _CACHE:
        _CACHE[key] = _build_program(lay["Lg"], batches1, batches2)
    nc = _CACHE[key]
    res = run_bass_kernel_spmd(nc, per_core, core_ids=list(range(NCORES)))
    out = np.empty((N, OUT), np.float32)
    for k in range(NCORES):
        out[k * NSH + lay["perms"][k]] = res.results[k]["out"][:NSH]
    return out


if __name__ == "__main__":
    d = np.load("/root/problem/_inp_check.npz")
    o = kernel(**{k: d[k] for k in d.files})
    ref = np.load("/root/problem/_ref_check.npy")
    rel = np.linalg.norm(o - ref) / np.linalg.norm(ref)
    err = np.abs(o - ref) / (np.abs(ref) + 1e-5)
    print("fro rel err:", rel, "max elem rel err:", err.max())
